# revision 11
# baseline (speedup 1.0000x reference)
"""Multi-head attention (B=2, S=4096, D=512, H=8, HD=64, fp32) on 8 TRN2 cores.

The softmax exp -- the original bottleneck (ScalarE ~252us busy of a ~313us
kernel) -- is split between ScalarE (true exp) and the Vector engine
(single-instruction Schraudolph exp2: i16 = rint(s*log2e*128 + 16250.5)
written through a bf16 tile's int16 bitcast view; the bf16 bit pattern IS
2^(s*log2e) to +-3.2%).  The softmax denominator is computed from the same
approximated values (ones column in v_sb), so the sawtooth's mean cancels;
measured output error 0.0046 vs the 2e-2 gate.  HW-validated: the DVE
f32->int16 conversion rounds to nearest (99.93% bit-exact vs rint).

Attention loop (CoreSim: PE 243.8us busy = 96-98% of the 247.6us marginal /
254.2us single-shot):
  - per kc chunk, BOTH heads' score matmuls are emitted back-to-back: their
    contraction rows (PE rows 0-63 / 64-127) are disjoint, so hardware that
    overlaps row-disjoint matmuls runs them concurrently (cost-model
    neutral, free upside on HW).
  - exp per (kc, head): head0 -> ScalarE, head1 -> DVE, with a few head1
    units flipped back to ScalarE to balance (r_dve); PSUM score tiles
    [128, 512] in a 6-deep rotation keep both engines fed.
  - last two kc run [h0,h0,h1,h1] with engines [S,S,D,D] so each head's
    av->SBUF copy (outt: h0 ScalarE, h1 DVE) never queues behind the other
    engine's exps at the q-group boundary.
  - bv is folded into the tail (out = attn@v/denom + bv via one DVE
    scalar_tensor_tensor) instead of pre-added into V; V writeback is a
    plain PSUM->SBUF bf16 copy on DVE, emitted lazily inside q-group 0.
  - the previous q-group's transpose/normalize/store tail is deferred
    tail_lag steps INTO the next q-group and spread one 128-row block per 3
    steps (no DVE burst); weights load as one combined DMA
    per projection (rearranged DRAM AP) on the Activation HWDGE queue while
    xT streams on the SP queue; qt/kt/v_sb are double-buffered by body
    parity (removes the WAR serialization between repeated bodies).

Sharding: core c -> batch b = c//4, head pair hp = c%4; attention is
head-independent so there is no cross-core communication.
"""

import numpy as np

B, S, D, H = 2, 4096, 512, 8
HD = D // H          # 64
OD = 128             # output dims per core (2 heads)
QW = 512             # query group width

LOG2E_128 = 128.0 / float(np.log(2.0))
BMAGIC = 16250.5     # 127*128 - 5.5 (sawtooth centering, round-to-nearest)

_CACHE = {}


def _build(s=S, rep=1, r_dve=0.42, bias_eng="S", wb_pat="D", tail_lag=8,
           outt_eng="SD", gcols=512, proj_at=2):
    import concourse.bacc as bacc
    import concourse.mybir as mybir
    import concourse.tile as tile

    f32 = mybir.dt.float32
    bf16 = mybir.dt.bfloat16
    i16 = mybir.dt.int16
    Exp = mybir.ActivationFunctionType.Exp
    Copy = mybir.ActivationFunctionType.Copy
    Ident = mybir.ActivationFunctionType.Identity
    Mult = mybir.AluOpType.mult
    Add = mybir.AluOpType.add

    qg_n = s // QW
    kc_n = s // 128
    sb_n = s // QW
    upg = gcols // QW     # units (kc,h) per exp group: 1 or 2
    grp_n = 2 * kc_n // upg
    n_dve = int(round(grp_n * r_dve))

    nc = bacc.Bacc(None, target_bir_lowering=False)

    xT = nc.dram_tensor("xT", [D, s], bf16, kind="ExternalInput")
    wqT = nc.dram_tensor("wqT", [D, OD], bf16, kind="ExternalInput")
    wkT = nc.dram_tensor("wkT", [D, OD], bf16, kind="ExternalInput")
    wvT = nc.dram_tensor("wvT", [D, OD], bf16, kind="ExternalInput")
    bq = nc.dram_tensor("bq", [OD, 1], f32, kind="ExternalInput")
    bk = nc.dram_tensor("bk", [OD, 1], f32, kind="ExternalInput")
    bvb = nc.dram_tensor("bvb", [128, OD], f32, kind="ExternalInput")
    ident = nc.dram_tensor("ident", [128, 128], f32, kind="ExternalInput")
    out = nc.dram_tensor("out", [s, OD], f32, kind="ExternalOutput")

    with tile.TileContext(nc) as tc:
        with (
            tc.tile_pool(name="persist", bufs=1) as persist,
            tc.tile_pool(name="exps", bufs=6) as exps,
            tc.tile_pool(name="outsb", bufs=8) as outsb,
            tc.tile_pool(name="outt", bufs=2) as outtp,
            tc.tile_pool(name="psbig", bufs=(3 if gcols == 1024 else 6),
                         space="PSUM") as psbig,
            tc.tile_pool(name="pssmall", bufs=2, space="PSUM") as pssmall,
        ):
            # Warm the ScalarE exp table immediately (ACT_TABLE_LOAD overlaps
            # the input DMAs instead of delaying the first real exp).
            warm = persist.tile([1, 1], f32, name="warm", tag="warm")
            nc.vector.memset(warm[:], 0.0)
            nc.scalar.activation(warm[:], warm[:], Exp)

            xt_all = persist.tile([128, 4, s], bf16, name="xt_all",
                                  tag="xt")
            xt = [xt_all[:, c, :] for c in range(4)]
            wk_a = persist.tile([128, 4, OD], bf16, name="wk_a", tag="wk")
            wq_a = persist.tile([128, 4, OD], bf16, name="wq_a", tag="wq")
            wv_a = persist.tile([128, 4, OD], bf16, name="wv_a", tag="wv")
            wq = [wq_a[:, c, :] for c in range(4)]
            wk = [wk_a[:, c, :] for c in range(4)]
            wv = [wv_a[:, c, :] for c in range(4)]
            bq_t = persist.tile([OD, 1], f32, name="bq_t", tag="bq")
            bk_t = persist.tile([OD, 1], f32, name="bk_t", tag="bk")
            bvb_t = persist.tile([128, OD], f32, name="bvb_t", tag="bvb")
            id_t = persist.tile([128, 128], f32, name="id_t", tag="ident")
            # Two HWDGE queues. The first projection block is ONE combined
            # DMA (all 4 x-chunks via a rearranged DRAM AP) on the SP queue
            # while wk loads on the Activation queue, so the first matmul's
            # inputs land in ~2us; the xT bulk streams after.
            nc.scalar.dma_start(
                wk_a[:], wkT[:].rearrange("(c p) j -> p c j", p=128))
            for c in (0, 1):
                nc.sync.dma_start(xt[c][:, 0:QW],
                                  xT[c * 128:(c + 1) * 128, 0:QW])
            for c in (2, 3):
                nc.scalar.dma_start(xt[c][:, 0:QW],
                                    xT[c * 128:(c + 1) * 128, 0:QW])
            nc.sync.dma_start(bk_t[:], bk[:])
            nc.scalar.dma_start(
                wq_a[:], wqT[:].rearrange("(c p) j -> p c j", p=128))
            nc.scalar.dma_start(bq_t[:], bq[:])
            nc.scalar.dma_start(
                wv_a[:], wvT[:].rearrange("(c p) j -> p c j", p=128))
            nc.scalar.dma_start(bvb_t[:], bvb[:])
            nc.scalar.dma_start(id_t[:], ident[:])
            h2 = (s - QW) // 2 + QW
            for lo, hi in ((QW, h2), (h2, s)):
                for c in range(4):
                    nc.sync.dma_start(xt[c][:, lo:hi],
                                      xT[c * 128:(c + 1) * 128, lo:hi])

            # Double-buffered by body parity: repeated bodies otherwise
            # serialize on the WAR hazard (body N+1's projections overwrite
            # qt/kt/v_sb while body N's last q-group still reads them).
            qt2 = [persist.tile([128, s], bf16, name=f"qt{p}", tag=f"qt{p}")
                   for p in (0, 1)]
            kt2 = [persist.tile([128, s], bf16, name=f"kt{p}", tag=f"kt{p}")
                   for p in (0, 1)]
            v_sb2 = [[persist.tile([128, kc_n * 65], bf16,
                                   name=f"vsb{h}_{p}", tag=f"vsb{h}_{p}")
                      for h in (0, 1)] for p in (0, 1)]

            for r in range(rep):
                p = r % 2
                ctx = dict(nc=nc, mybir=mybir, s=s, qt=qt2[p], kt=kt2[p],
                           xt=xt, wq=wq, wk=wk, wv=wv, bq_t=bq_t, bk_t=bk_t,
                           bvb_t=bvb_t, id_t=id_t, v_sb=v_sb2[p], out=out,
                           exps=exps, outsb=outsb, outtp=outtp,
                           psbig=psbig, pssmall=pssmall,
                           n_dve=n_dve, bias_eng=bias_eng, wb_pat=wb_pat,
                           tail_lag=tail_lag, outt_eng=outt_eng, gcols=gcols,
                           proj_at=proj_at,
                           f32=f32, bf16=bf16, i16=i16, Exp=Exp, Copy=Copy,
                           Ident=Ident, Mult=Mult, Add=Add)
                _emit_body(ctx)

    nc.compile()
    return nc


def _emit_body(c):
    nc = c["nc"]
    s = c["s"]
    f32, bf16, i16 = c["f32"], c["bf16"], c["i16"]
    Exp, Copy, Ident, Mult, Add = (c["Exp"], c["Copy"], c["Ident"],
                                   c["Mult"], c["Add"])
    qt, kt, xt, v_sb = c["qt"], c["kt"], c["xt"], c["v_sb"]
    wq, wk, wv = c["wq"], c["wk"], c["wv"]
    bq_t, bk_t, bvb_t, id_t = c["bq_t"], c["bk_t"], c["bvb_t"], c["id_t"]
    out = c["out"]
    exps, outsb, outtp = c["exps"], c["outsb"], c["outtp"]
    psbig, pssmall = c["psbig"], c["pssmall"]
    n_dve, bias_eng, wb_pat = c["n_dve"], c["bias_eng"], c["wb_pat"]
    tail_lag, outt_eng = c["tail_lag"], c["outt_eng"]
    gcols = c["gcols"]

    qg_n = s // QW
    kc_n = s // 128
    sb_n = s // QW
    upg = gcols // QW
    grp_n = 2 * kc_n // upg

    # Exp engine assignment per (kc, head): head0 -> ScalarE, head1 -> DVE
    # (concurrent engines per kc pair); `flip_s` of the head1 units are
    # flipped back to ScalarE to fine-tune the load split.  The last two kc
    # run [h0,h0,h1,h1] with engines [S,S,D,D] so each head's av->SBUF copy
    # (outt: h0 ScalarE, h1 DVE) never queues behind the other engine.
    flip_s = max(0, 32 - n_dve) if upg == 1 else 0
    flip_set = set()
    acc = 0
    for kc2 in range(2, kc_n - 2):
        acc += flip_s
        if acc >= kc_n - 4:
            acc -= kc_n - 4
            flip_set.add(kc2)

    def exp_eng(kc2, h):
        if kc2 >= kc_n - 2:
            return "S" if h == 0 else "D"
        if h == 0:
            return "S"
        return "S" if kc2 in flip_set else "D"

    def proj_qk(dst, w, b_t, sb):
        ps = psbig.tile([128, QW], f32, name="ps_proj", tag="sc")
        for cc in range(4):
            nc.tensor.matmul(
                ps[:, 0:QW],
                lhsT=w[cc][:],
                rhs=xt[cc][:, sb * QW:(sb + 1) * QW],
                start=(cc == 0),
                stop=(cc == 3),
            )
        dslice = dst[:, sb * QW:(sb + 1) * QW]
        if bias_eng == "S":
            nc.scalar.activation(dslice, ps[:, 0:QW], Ident, bias=b_t[:])
        else:
            nc.vector.tensor_scalar_add(dslice, ps[:, 0:QW], b_t[:])

    # K fully, then Q block 0 (enough to start attention qg 0)
    for sb in range(sb_n):
        proj_qk(kt, wk, bk_t, sb)
    proj_qk(qt, wq, bq_t, 0)

    # ---- V projection: pure V (bv folded into the tail), ones col 64.
    # Emitted lazily, interleaved into q-group 0's unit loop (chunk kc lands
    # a few groups before unit (kc, h) consumes it) so the PE isn't serial
    # on V while the exp engines sit idle.
    for h in (0, 1):
        nc.vector.memset(v_sb[h][:], 1.0)

    vp_next = [0]

    def emit_vp_upto(kc_needed):
        while vp_next[0] <= min(kc_needed, kc_n - 1):
            sb = vp_next[0]
            ps = psbig.tile([128, 128], f32, name="ps_vp", tag="sc")
            for cc in range(4):
                nc.tensor.matmul(
                    ps[:],
                    lhsT=xt[cc][:, sb * 128:(sb + 1) * 128],
                    rhs=wv[cc][:],
                    start=(cc == 0),
                    stop=(cc == 3),
                )
            for h in (0, 1):
                dst = v_sb[h][:, sb * 65:sb * 65 + 64]
                src = ps[:, h * 64:(h + 1) * 64]
                eng = wb_pat[(2 * sb + h) % len(wb_pat)]
                if eng == "S":
                    nc.scalar.activation(dst, src, Copy)
                else:
                    nc.vector.tensor_copy(dst, src)
            vp_next[0] += 1

    # ---- attention ----
    # Tail split: the av->SBUF copies (which free the av PSUM slots for the
    # next q-group) are emitted right after the q-group's last attnV; the
    # transpose/normalize/store half is deferred into the next q-group so
    # both exp engines stay fed across the boundary.
    def emit_tail_block(qg, outts, blk):
        ot = outsb.tile([128, OD], f32, name="ot", tag="outsb")
        for h in (0, 1):
            tp = psbig.tile([128, 65], f32, name="tp", tag="sc")
            nc.tensor.transpose(
                tp[:],
                outts[h][:, blk * 128:(blk + 1) * 128],
                id_t[0:65, 0:65],
            )
            rs = outsb.tile([128, 1], f32, name="rs", tag="rs")
            nc.vector.reciprocal(rs[:], tp[:, 64:65])
            nc.vector.scalar_tensor_tensor(
                ot[:, h * HD:(h + 1) * HD],
                tp[:, 0:64], rs[:],
                bvb_t[:, h * HD:(h + 1) * HD],
                Mult, Add,
            )
        r0 = qg * QW + blk * 128
        nc.sync.dma_start(out[r0:r0 + 128, :], ot[:])

    def emit_tail_rest(qg, outts):
        for blk in range(4):
            emit_tail_block(qg, outts, blk)

    def emit_tail_half(qg, outt, h):
        # Last q-group only: per-head eager tail with half-width stores so
        # head 0's drain overlaps head 1's final units.
        for blk in range(4):
            tp = psbig.tile([128, 65], f32, name="tp", tag="sc")
            nc.tensor.transpose(
                tp[:],
                outt[:, blk * 128:(blk + 1) * 128],
                id_t[0:65, 0:65],
            )
            rs = outsb.tile([128, 1], f32, name="rs", tag="rs")
            oth = outsb.tile([128, HD], f32, name="oth", tag="outsb")
            nc.vector.reciprocal(rs[:], tp[:, 64:65])
            nc.vector.scalar_tensor_tensor(
                oth[:], tp[:, 0:64], rs[:],
                bvb_t[:, h * HD:(h + 1) * HD],
                Mult, Add,
            )
            r0 = qg * QW + blk * 128
            nc.sync.dma_start(
                out[r0:r0 + 128, h * HD:(h + 1) * HD], oth[:])

    def emit_one_outt(av, h):
        outt = outtp.tile([65, QW], f32, name="outt", tag="outt")
        if outt_eng[h] == "S":
            nc.scalar.activation(outt[:], av[h][:], Copy)
        else:
            nc.vector.tensor_copy(outt[:], av[h][:])
        return outt

    pending_tail = None
    for qg in range(qg_n):
        av = [pssmall.tile([65, QW], f32, name="av", tag="av")
              for _ in (0, 1)]
        outts = [None, None]
        # kc order: natural, except the last two kc run per-head
        # ([h0,h0,h1,h1]) so head0 closes early (see exp_eng docstring).
        plan = [(kc2, (0, 1)) for kc2 in range(kc_n - 2)]
        plan += [(kc_n - 2, (0,)), (kc_n - 1, (0,)),
                 (kc_n - 2, (1,)), (kc_n - 1, (1,))]
        for step, (kc2, heads) in enumerate(plan):
            if qg == 0:
                emit_vp_upto(kc2 + 3)
            if step == c.get("proj_at", 2) and qg + 1 < qg_n:
                proj_qk(qt, wq, bq_t, qg + 1)
            # Previous q-group's tail, one block per slot: spreads the
            # DVE recip/normalize burst across several steps.
            if (pending_tail is not None and step >= tail_lag
                    and (step - tail_lag) % 3 == 0):
                blk = (step - tail_lag) // 3
                emit_tail_block(pending_tail[0], pending_tail[1], blk)
                if blk == 3:
                    pending_tail = None
            pss = []
            for h in heads:
                ps = psbig.tile([128, QW], f32, name="ps_sc", tag="sc")
                nc.tensor.matmul(
                    ps[:],
                    lhsT=kt[h * HD:(h + 1) * HD,
                            kc2 * 128:(kc2 + 1) * 128],
                    rhs=qt[h * HD:(h + 1) * HD, qg * QW:(qg + 1) * QW],
                    start=True,
                    stop=True,
                )
                pss.append(ps)
            exs = []
            for ps, h in zip(pss, heads):
                ex = exps.tile([128, QW], bf16, name="ex", tag="exp")
                if exp_eng(kc2, h) == "D":
                    nc.vector.tensor_scalar(
                        ex[:].bitcast(i16), ps[:], LOG2E_128, BMAGIC,
                        Mult, Add)
                else:
                    nc.scalar.activation(ex[:], ps[:], Exp)
                exs.append(ex)
            for ex, h in zip(exs, heads):
                nc.tensor.matmul(
                    av[h][:],
                    lhsT=v_sb[h][:, kc2 * 65:kc2 * 65 + 65],
                    rhs=ex[:],
                    start=(kc2 == 0),
                    stop=(kc2 == kc_n - 1),
                )
                if kc2 == kc_n - 1:
                    outts[h] = emit_one_outt(av, h)
        if pending_tail is not None:
            emit_tail_rest(*pending_tail)
            pending_tail = None
        pending_tail = (qg, outts)
    emit_tail_rest(*pending_tail)
                pending_tail = None
            seq = unit_seq_last if qg == qg_n - 1 else unit_seq
            units = seq[g * upg:(g + 1) * upg]
            ps = psbig.tile([128, upg * QW], f32, name="ps_sc", tag="sc")
            for i, (kc, h) in enumerate(units):
                nc.tensor.matmul(
                    ps[:, i * QW:(i + 1) * QW],
                    lhsT=kt[h * HD:(h + 1) * HD, kc * 128:(kc + 1) * 128],
                    rhs=qt[h * HD:(h + 1) * HD, qg * QW:(qg + 1) * QW],
                    start=True,
                    stop=True,
                )
            ex = exps.tile([128, upg * QW], bf16, name="ex", tag="exp")
            if g in dve_set:
                nc.vector.tensor_scalar(
                    ex[:].bitcast(i16), ps[:], LOG2E_128, BMAGIC, Mult, Add)
            else:
                nc.scalar.activation(ex[:], ps[:], Exp)
            for i, (kc, h) in enumerate(units):
                nc.tensor.matmul(
                    av[h][:],
                    lhsT=v_sb[h][:, kc * 65:kc * 65 + 65],
                    rhs=ex[:, i * QW:(i + 1) * QW],
                    start=(kc == 0),
                    stop=(kc == kc_n - 1),
                )
                if kc == kc_n - 1:
                    outts[h] = emit_one_outt(av, h)
        if pending_tail is not None:
            emit_tail_rest(*pending_tail)
            pending_tail = None
        pending_tail = (qg, outts)
    emit_tail_rest(*pending_tail)


def _get_nc(s=S):
    if s not in _CACHE:
        _CACHE[s] = _build(s)
    return _CACHE[s]


def _shard_inputs(x, Wq, bq, Wk, bk, Wv, bv):
    import ml_dtypes

    bf16 = ml_dtypes.bfloat16
    f32 = np.float32
    ident = np.eye(128, dtype=f32)
    xTb = [np.ascontiguousarray(x[b].T).astype(bf16) for b in range(B)]
    wq_s, wk_s, wv_s, bq_s, bk_s, bvb_s = [], [], [], [], [], []
    for hp in range(4):
        r = slice(128 * hp, 128 * hp + 128)
        wq_s.append(np.ascontiguousarray((Wq[r] * 0.125).T).astype(bf16))
        wk_s.append(np.ascontiguousarray(Wk[r].T).astype(bf16))
        wv_s.append(np.ascontiguousarray(Wv[r].T).astype(bf16))
        bq_s.append((bq[r] * 0.125).reshape(128, 1).astype(f32))
        bk_s.append(bk[r].reshape(128, 1).astype(f32))
        bvb_s.append(np.tile(bv[r][None, :], (128, 1)).astype(f32))
    in_maps = []
    for c in range(8):
        b, hp = divmod(c, 4)
        in_maps.append({
            "xT": xTb[b],
            "wqT": wq_s[hp],
            "wkT": wk_s[hp],
            "wvT": wv_s[hp],
            "bq": bq_s[hp],
            "bk": bk_s[hp],
            "bvb": bvb_s[hp],
            "ident": ident,
        })
    return in_maps


def kernel(x, Wq, bq, Wk, bk, Wv, bv, _trace=False):
    from concourse.bass_utils import run_bass_kernel_spmd

    x = np.asarray(x, dtype=np.float32)
    Wq = np.asarray(Wq, dtype=np.float32)
    bq = np.asarray(bq, dtype=np.float32)
    Wk = np.asarray(Wk, dtype=np.float32)
    bk = np.asarray(bk, dtype=np.float32)
    Wv = np.asarray(Wv, dtype=np.float32)
    bv = np.asarray(bv, dtype=np.float32)

    nc = _get_nc(S)
    in_maps = _shard_inputs(x, Wq, bq, Wk, bk, Wv, bv)
    try:
        res = run_bass_kernel_spmd(nc, in_maps, core_ids=list(range(8)),
                                   trace=_trace)
    except (ModuleNotFoundError, ImportError):
        # Tracing was requested (arg or BASS_TRACE env) but this axon client
        # has no NTFF profiling hooks -- rerun untraced.
        import os
        os.environ["BASS_NEVER_TRACE"] = "1"
        res = run_bass_kernel_spmd(nc, in_maps, core_ids=list(range(8)),
                                   trace=False)
    kernel._last_results = res

    out = np.empty((B, S, D), dtype=np.float32)
    for c in range(8):
        b, hp = divmod(c, 4)
        out[b, :, 128 * hp:128 * hp + 128] = res.results[c]["out"]
    return out


# revision 12
# speedup vs baseline: 1.0010x; 1.0010x over previous
"""Multi-head attention (B=2, S=4096, D=512, H=8, HD=64, fp32) on 8 TRN2 cores.

The softmax exp -- the original bottleneck (ScalarE ~252us busy of a ~313us
kernel) -- is split between ScalarE (true exp) and the Vector engine
(single-instruction Schraudolph exp2: i16 = rint(s*log2e*128 + 16250.5)
written through a bf16 tile's int16 bitcast view; the bf16 bit pattern IS
2^(s*log2e) to +-3.2%).  The softmax denominator is computed from the same
approximated values (ones column in v_sb), so the sawtooth's mean cancels;
measured output error 0.0046 vs the 2e-2 gate.  HW-validated: the DVE
f32->int16 conversion rounds to nearest (99.93% bit-exact vs rint).

Attention loop (CoreSim: PE 243.8us busy = 96-98% of the 247.6us marginal /
254.2us single-shot):
  - per kc chunk, BOTH heads' score matmuls are emitted back-to-back: their
    contraction rows (PE rows 0-63 / 64-127) are disjoint, so hardware that
    overlaps row-disjoint matmuls runs them concurrently (cost-model
    neutral, free upside on HW).
  - exp per (kc, head): head0 -> ScalarE, head1 -> DVE, with a few head1
    units flipped back to ScalarE to balance (r_dve); PSUM score tiles
    [128, 512] in a 6-deep rotation keep both engines fed.
  - last two kc run [h0,h0,h1,h1] with engines [S,S,D,D] so each head's
    av->SBUF copy (outt: h0 ScalarE, h1 DVE) never queues behind the other
    engine's exps at the q-group boundary.
  - bv is folded into the tail (out = attn@v/denom + bv via one DVE
    scalar_tensor_tensor) instead of pre-added into V; V writeback is a
    plain PSUM->SBUF bf16 copy on DVE, emitted lazily inside q-group 0.
  - the previous q-group's transpose/normalize/store tail is deferred
    tail_lag steps INTO the next q-group and spread one 128-row block per 3
    steps (no DVE burst); weights load as one combined DMA
    per projection (rearranged DRAM AP) on the Activation HWDGE queue while
    xT streams on the SP queue; qt/kt/v_sb are double-buffered by body
    parity (removes the WAR serialization between repeated bodies).

Sharding: core c -> batch b = c//4, head pair hp = c%4; attention is
head-independent so there is no cross-core communication.
"""

import numpy as np

B, S, D, H = 2, 4096, 512, 8
HD = D // H          # 64
OD = 128             # output dims per core (2 heads)
QW = 512             # query group width

LOG2E_128 = 128.0 / float(np.log(2.0))
BMAGIC = 16250.5     # 127*128 - 5.5 (sawtooth centering, round-to-nearest)

_CACHE = {}


def _build(s=S, rep=1, r_dve=0.42, bias_eng="S", wb_pat="D", tail_lag=8,
           outt_eng="SD", gcols=512, proj_at=2):
    import concourse.bacc as bacc
    import concourse.mybir as mybir
    import concourse.tile as tile

    f32 = mybir.dt.float32
    bf16 = mybir.dt.bfloat16
    i16 = mybir.dt.int16
    Exp = mybir.ActivationFunctionType.Exp
    Copy = mybir.ActivationFunctionType.Copy
    Ident = mybir.ActivationFunctionType.Identity
    Mult = mybir.AluOpType.mult
    Add = mybir.AluOpType.add

    qg_n = s // QW
    kc_n = s // 128
    sb_n = s // QW
    upg = gcols // QW     # units (kc,h) per exp group: 1 or 2
    grp_n = 2 * kc_n // upg
    n_dve = int(round(grp_n * r_dve))

    nc = bacc.Bacc(None, target_bir_lowering=False)

    xT = nc.dram_tensor("xT", [D, s], bf16, kind="ExternalInput")
    wqT = nc.dram_tensor("wqT", [D, OD], bf16, kind="ExternalInput")
    wkT = nc.dram_tensor("wkT", [D, OD], bf16, kind="ExternalInput")
    wvT = nc.dram_tensor("wvT", [D, OD], bf16, kind="ExternalInput")
    bq = nc.dram_tensor("bq", [OD, 1], f32, kind="ExternalInput")
    bk = nc.dram_tensor("bk", [OD, 1], f32, kind="ExternalInput")
    bvb = nc.dram_tensor("bvb", [128, OD], f32, kind="ExternalInput")
    ident = nc.dram_tensor("ident", [128, 128], f32, kind="ExternalInput")
    out = nc.dram_tensor("out", [s, OD], f32, kind="ExternalOutput")

    with tile.TileContext(nc) as tc:
        with (
            tc.tile_pool(name="persist", bufs=1) as persist,
            tc.tile_pool(name="exps", bufs=6) as exps,
            tc.tile_pool(name="outsb", bufs=8) as outsb,
            tc.tile_pool(name="outt", bufs=2) as outtp,
            tc.tile_pool(name="psbig", bufs=(3 if gcols == 1024 else 6),
                         space="PSUM") as psbig,
            tc.tile_pool(name="pssmall", bufs=2, space="PSUM") as pssmall,
        ):
            xt_all = persist.tile([128, 4, s], bf16, name="xt_all",
                                  tag="xt")
            xt = [xt_all[:, c, :] for c in range(4)]
            wk_a = persist.tile([128, 4, OD], bf16, name="wk_a", tag="wk")
            wq_a = persist.tile([128, 4, OD], bf16, name="wq_a", tag="wq")
            wv_a = persist.tile([128, 4, OD], bf16, name="wv_a", tag="wv")
            wq = [wq_a[:, c, :] for c in range(4)]
            wk = [wk_a[:, c, :] for c in range(4)]
            wv = [wv_a[:, c, :] for c in range(4)]
            bq_t = persist.tile([OD, 1], f32, name="bq_t", tag="bq")
            bk_t = persist.tile([OD, 1], f32, name="bk_t", tag="bk")
            bvb_t = persist.tile([128, OD], f32, name="bvb_t", tag="bvb")
            id_t = persist.tile([128, 128], f32, name="id_t", tag="ident")
            # Two HWDGE queues. The first projection block is ONE combined
            # DMA (all 4 x-chunks via a rearranged DRAM AP) on the SP queue
            # while wk loads on the Activation queue, so the first matmul's
            # inputs land in ~2us; the xT bulk streams after.
            # Warm the ScalarE exp table early: the
            # ~1.3us ACT_TABLE_LOAD otherwise delays the issue (both run on
            # the Activation engine) and with it the first matmul.
            warm = persist.tile([1, 1], f32, name="warm", tag="warm")
            nc.vector.memset(warm[:], 0.0)
            nc.scalar.activation(warm[:], warm[:], Exp)
            # All four chunks' first block as ONE 3D-AP DMA (one issue
            # overhead instead of four); the bulk stays fine-grained so the
            # K projection's consumption order is fed progressively.
            xTr = xT[:].rearrange("(c p) j -> p c j", p=128)
            nc.scalar.dma_start(
                wk_a[:], wkT[:].rearrange("(c p) j -> p c j", p=128))
            nc.sync.dma_start(xt_all[:, :, 0:QW], xTr[:, :, 0:QW])
            nc.sync.dma_start(bk_t[:], bk[:])
            nc.scalar.dma_start(
                wq_a[:], wqT[:].rearrange("(c p) j -> p c j", p=128))
            nc.scalar.dma_start(bq_t[:], bq[:])
            nc.scalar.dma_start(
                wv_a[:], wvT[:].rearrange("(c p) j -> p c j", p=128))
            nc.scalar.dma_start(bvb_t[:], bvb[:])
            nc.scalar.dma_start(id_t[:], ident[:])
            h2 = (s - QW) // 2 + QW
            for lo, hi in ((QW, h2), (h2, s)):
                for c in range(4):
                    nc.sync.dma_start(xt[c][:, lo:hi],
                                      xT[c * 128:(c + 1) * 128, lo:hi])

            # Double-buffered by body parity: repeated bodies otherwise
            # serialize on the WAR hazard (body N+1's projections overwrite
            # qt/kt/v_sb while body N's last q-group still reads them).
            qt2 = [persist.tile([128, s], bf16, name=f"qt{p}", tag=f"qt{p}")
                   for p in (0, 1)]
            kt2 = [persist.tile([128, s], bf16, name=f"kt{p}", tag=f"kt{p}")
                   for p in (0, 1)]
            v_sb2 = [[persist.tile([128, kc_n * 65], bf16,
                                   name=f"vsb{h}_{p}", tag=f"vsb{h}_{p}")
                      for h in (0, 1)] for p in (0, 1)]

            for r in range(rep):
                p = r % 2
                ctx = dict(nc=nc, mybir=mybir, s=s, qt=qt2[p], kt=kt2[p],
                           xt=xt, wq=wq, wk=wk, wv=wv, bq_t=bq_t, bk_t=bk_t,
                           bvb_t=bvb_t, id_t=id_t, v_sb=v_sb2[p], out=out,
                           exps=exps, outsb=outsb, outtp=outtp,
                           psbig=psbig, pssmall=pssmall,
                           n_dve=n_dve, bias_eng=bias_eng, wb_pat=wb_pat,
                           tail_lag=tail_lag, outt_eng=outt_eng, gcols=gcols,
                           proj_at=proj_at,
                           f32=f32, bf16=bf16, i16=i16, Exp=Exp, Copy=Copy,
                           Ident=Ident, Mult=Mult, Add=Add)
                _emit_body(ctx)

    nc.compile()
    return nc


def _emit_body(c):
    nc = c["nc"]
    s = c["s"]
    f32, bf16, i16 = c["f32"], c["bf16"], c["i16"]
    Exp, Copy, Ident, Mult, Add = (c["Exp"], c["Copy"], c["Ident"],
                                   c["Mult"], c["Add"])
    qt, kt, xt, v_sb = c["qt"], c["kt"], c["xt"], c["v_sb"]
    wq, wk, wv = c["wq"], c["wk"], c["wv"]
    bq_t, bk_t, bvb_t, id_t = c["bq_t"], c["bk_t"], c["bvb_t"], c["id_t"]
    out = c["out"]
    exps, outsb, outtp = c["exps"], c["outsb"], c["outtp"]
    psbig, pssmall = c["psbig"], c["pssmall"]
    n_dve, bias_eng, wb_pat = c["n_dve"], c["bias_eng"], c["wb_pat"]
    tail_lag, outt_eng = c["tail_lag"], c["outt_eng"]
    gcols = c["gcols"]

    qg_n = s // QW
    kc_n = s // 128
    sb_n = s // QW
    upg = gcols // QW
    grp_n = 2 * kc_n // upg

    # Exp engine assignment per (kc, head): head0 -> ScalarE, head1 -> DVE
    # (concurrent engines per kc pair); `flip_s` of the head1 units are
    # flipped back to ScalarE to fine-tune the load split.  The last two kc
    # run [h0,h0,h1,h1] with engines [S,S,D,D] so each head's av->SBUF copy
    # (outt: h0 ScalarE, h1 DVE) never queues behind the other engine.
    flip_s = max(0, 32 - n_dve) if upg == 1 else 0
    flip_set = set()
    acc = 0
    for kc2 in range(2, kc_n - 2):
        acc += flip_s
        if acc >= kc_n - 4:
            acc -= kc_n - 4
            flip_set.add(kc2)

    def exp_eng(kc2, h):
        if kc2 >= kc_n - 2:
            return "S" if h == 0 else "D"
        if h == 0:
            return "S"
        return "S" if kc2 in flip_set else "D"

    def proj_qk(dst, w, b_t, sb, narrow=False):
        ps = psbig.tile([128, QW], f32, name="ps_proj", tag="sc")
        # narrow: 128-col sub-blocks so the very first matmul only needs the
        # first 128 x columns (cuts the prologue DMA wait).
        subs = 4 if narrow else 1
        sw = QW // subs
        for sub in range(subs):
            for cc in range(4):
                nc.tensor.matmul(
                    ps[:, sub * sw:(sub + 1) * sw],
                    lhsT=w[cc][:],
                    rhs=xt[cc][:, sb * QW + sub * sw:sb * QW + (sub + 1) * sw],
                    start=(cc == 0),
                    stop=(cc == 3),
                )
        dslice = dst[:, sb * QW:(sb + 1) * QW]
        if bias_eng == "S":
            nc.scalar.activation(dslice, ps[:, 0:QW], Ident, bias=b_t[:])
        else:
            nc.vector.tensor_scalar_add(dslice, ps[:, 0:QW], b_t[:])

    # K fully, then Q block 0 (enough to start attention qg 0)
    for sb in range(sb_n):
        proj_qk(kt, wk, bk_t, sb)
    proj_qk(qt, wq, bq_t, 0)

    # ---- V projection: pure V (bv folded into the tail), ones col 64.
    # Emitted lazily, interleaved into q-group 0's unit loop (chunk kc lands
    # a few groups before unit (kc, h) consumes it) so the PE isn't serial
    # on V while the exp engines sit idle.
    for h in (0, 1):
        nc.vector.memset(v_sb[h][:], 1.0)

    vp_next = [0]

    def emit_vp_upto(kc_needed):
        while vp_next[0] <= min(kc_needed, kc_n - 1):
            sb = vp_next[0]
            ps = psbig.tile([128, 128], f32, name="ps_vp", tag="sc")
            for cc in range(4):
                nc.tensor.matmul(
                    ps[:],
                    lhsT=xt[cc][:, sb * 128:(sb + 1) * 128],
                    rhs=wv[cc][:],
                    start=(cc == 0),
                    stop=(cc == 3),
                )
            for h in (0, 1):
                dst = v_sb[h][:, sb * 65:sb * 65 + 64]
                src = ps[:, h * 64:(h + 1) * 64]
                eng = wb_pat[(2 * sb + h) % len(wb_pat)]
                if eng == "S":
                    nc.scalar.activation(dst, src, Copy)
                else:
                    nc.vector.tensor_copy(dst, src)
            vp_next[0] += 1

    # ---- attention ----
    # Tail split: the av->SBUF copies (which free the av PSUM slots for the
    # next q-group) are emitted right after the q-group's last attnV; the
    # transpose/normalize/store half is deferred into the next q-group so
    # both exp engines stay fed across the boundary.
    def emit_tail_block(qg, outts, blk, alt_q=False):
        ot = outsb.tile([128, OD], f32, name="ot", tag="outsb")
        for h in (0, 1):
            tp = psbig.tile([128, 65], f32, name="tp", tag="sc")
            nc.tensor.transpose(
                tp[:],
                outts[h][:, blk * 128:(blk + 1) * 128],
                id_t[0:65, 0:65],
            )
            rs = outsb.tile([128, 1], f32, name="rs", tag="rs")
            nc.vector.reciprocal(rs[:], tp[:, 64:65])
            nc.vector.scalar_tensor_tensor(
                ot[:, h * HD:(h + 1) * HD],
                tp[:, 0:64], rs[:],
                bvb_t[:, h * HD:(h + 1) * HD],
                Mult, Add,
            )
        r0 = qg * QW + blk * 128
        # alt_q: only for the kernel-final tail, when the Activation queue
        # is idle (mid-kernel its DMA issues would block ScalarE exps).
        eng = nc.scalar if (alt_q and blk % 2 == 1) else nc.sync
        eng.dma_start(out[r0:r0 + 128, :], ot[:])

    def emit_tail_rest(qg, outts, alt_q=False):
        for blk in range(4):
            emit_tail_block(qg, outts, blk, alt_q=alt_q)

    def emit_tail_half(qg, outt, h):
        # Last q-group only: per-head eager tail with half-width stores so
        # head 0's drain overlaps head 1's final units.
        for blk in range(4):
            tp = psbig.tile([128, 65], f32, name="tp", tag="sc")
            nc.tensor.transpose(
                tp[:],
                outt[:, blk * 128:(blk + 1) * 128],
                id_t[0:65, 0:65],
            )
            rs = outsb.tile([128, 1], f32, name="rs", tag="rs")
            oth = outsb.tile([128, HD], f32, name="oth", tag="outsb")
            nc.vector.reciprocal(rs[:], tp[:, 64:65])
            nc.vector.scalar_tensor_tensor(
                oth[:], tp[:, 0:64], rs[:],
                bvb_t[:, h * HD:(h + 1) * HD],
                Mult, Add,
            )
            r0 = qg * QW + blk * 128
            nc.sync.dma_start(
                out[r0:r0 + 128, h * HD:(h + 1) * HD], oth[:])

    def emit_one_outt(av, h):
        outt = outtp.tile([65, QW], f32, name="outt", tag="outt")
        if outt_eng[h] == "S":
            nc.scalar.activation(outt[:], av[h][:], Copy)
        else:
            nc.vector.tensor_copy(outt[:], av[h][:])
        return outt

    pending_tail = None
    for qg in range(qg_n):
        av = [pssmall.tile([65, QW], f32, name="av", tag="av")
              for _ in (0, 1)]
        outts = [None, None]
        # kc order: natural, except the last two kc run per-head
        # ([h0,h0,h1,h1]) so head0 closes early (see exp_eng docstring).
        plan = [(kc2, (0, 1)) for kc2 in range(kc_n - 2)]
        plan += [(kc_n - 2, (0,)), (kc_n - 1, (0,)),
                 (kc_n - 2, (1,)), (kc_n - 1, (1,))]
        for step, (kc2, heads) in enumerate(plan):
            if qg == 0:
                emit_vp_upto(kc2 + 3)
            if step == c.get("proj_at", 2) and qg + 1 < qg_n:
                proj_qk(qt, wq, bq_t, qg + 1)
            # Previous q-group's tail, one block per slot: spreads the
            # DVE recip/normalize burst across several steps.
            if (pending_tail is not None and step >= tail_lag
                    and (step - tail_lag) % 3 == 0):
                blk = (step - tail_lag) // 3
                emit_tail_block(pending_tail[0], pending_tail[1], blk)
                if blk == 3:
                    pending_tail = None
            pss = []
            for h in heads:
                ps = psbig.tile([128, QW], f32, name="ps_sc", tag="sc")
                nc.tensor.matmul(
                    ps[:],
                    lhsT=kt[h * HD:(h + 1) * HD,
                            kc2 * 128:(kc2 + 1) * 128],
                    rhs=qt[h * HD:(h + 1) * HD, qg * QW:(qg + 1) * QW],
                    start=True,
                    stop=True,
                )
                pss.append(ps)
            exs = []
            for ps, h in zip(pss, heads):
                ex = exps.tile([128, QW], bf16, name="ex", tag="exp")
                if exp_eng(kc2, h) == "D":
                    nc.vector.tensor_scalar(
                        ex[:].bitcast(i16), ps[:], LOG2E_128, BMAGIC,
                        Mult, Add)
                else:
                    nc.scalar.activation(ex[:], ps[:], Exp)
                exs.append(ex)
            for ex, h in zip(exs, heads):
                nc.tensor.matmul(
                    av[h][:],
                    lhsT=v_sb[h][:, kc2 * 65:kc2 * 65 + 65],
                    rhs=ex[:],
                    start=(kc2 == 0),
                    stop=(kc2 == kc_n - 1),
                )
                if kc2 == kc_n - 1:
                    outts[h] = emit_one_outt(av, h)
        if pending_tail is not None:
            emit_tail_rest(*pending_tail)
            pending_tail = None
        pending_tail = (qg, outts)
    emit_tail_rest(*pending_tail, alt_q=True)
                pending_tail = None
            seq = unit_seq_last if qg == qg_n - 1 else unit_seq
            units = seq[g * upg:(g + 1) * upg]
            ps = psbig.tile([128, upg * QW], f32, name="ps_sc", tag="sc")
            for i, (kc, h) in enumerate(units):
                nc.tensor.matmul(
                    ps[:, i * QW:(i + 1) * QW],
                    lhsT=kt[h * HD:(h + 1) * HD, kc * 128:(kc + 1) * 128],
                    rhs=qt[h * HD:(h + 1) * HD, qg * QW:(qg + 1) * QW],
                    start=True,
                    stop=True,
                )
            ex = exps.tile([128, upg * QW], bf16, name="ex", tag="exp")
            if g in dve_set:
                nc.vector.tensor_scalar(
                    ex[:].bitcast(i16), ps[:], LOG2E_128, BMAGIC, Mult, Add)
            else:
                nc.scalar.activation(ex[:], ps[:], Exp)
            for i, (kc, h) in enumerate(units):
                nc.tensor.matmul(
                    av[h][:],
                    lhsT=v_sb[h][:, kc * 65:kc * 65 + 65],
                    rhs=ex[:, i * QW:(i + 1) * QW],
                    start=(kc == 0),
                    stop=(kc == kc_n - 1),
                )
                if kc == kc_n - 1:
                    outts[h] = emit_one_outt(av, h)
        if pending_tail is not None:
            emit_tail_rest(*pending_tail)
            pending_tail = None
        pending_tail = (qg, outts)
    emit_tail_rest(*pending_tail, alt_q=True)


def _get_nc(s=S):
    if s not in _CACHE:
        _CACHE[s] = _build(s)
    return _CACHE[s]


def _shard_inputs(x, Wq, bq, Wk, bk, Wv, bv):
    import ml_dtypes

    bf16 = ml_dtypes.bfloat16
    f32 = np.float32
    ident = np.eye(128, dtype=f32)
    xTb = [np.ascontiguousarray(x[b].T).astype(bf16) for b in range(B)]
    wq_s, wk_s, wv_s, bq_s, bk_s, bvb_s = [], [], [], [], [], []
    for hp in range(4):
        r = slice(128 * hp, 128 * hp + 128)
        wq_s.append(np.ascontiguousarray((Wq[r] * 0.125).T).astype(bf16))
        wk_s.append(np.ascontiguousarray(Wk[r].T).astype(bf16))
        wv_s.append(np.ascontiguousarray(Wv[r].T).astype(bf16))
        bq_s.append((bq[r] * 0.125).reshape(128, 1).astype(f32))
        bk_s.append(bk[r].reshape(128, 1).astype(f32))
        bvb_s.append(np.tile(bv[r][None, :], (128, 1)).astype(f32))
    in_maps = []
    for c in range(8):
        b, hp = divmod(c, 4)
        in_maps.append({
            "xT": xTb[b],
            "wqT": wq_s[hp],
            "wkT": wk_s[hp],
            "wvT": wv_s[hp],
            "bq": bq_s[hp],
            "bk": bk_s[hp],
            "bvb": bvb_s[hp],
            "ident": ident,
        })
    return in_maps


def kernel(x, Wq, bq, Wk, bk, Wv, bv, _trace=False):
    from concourse.bass_utils import run_bass_kernel_spmd

    x = np.asarray(x, dtype=np.float32)
    Wq = np.asarray(Wq, dtype=np.float32)
    bq = np.asarray(bq, dtype=np.float32)
    Wk = np.asarray(Wk, dtype=np.float32)
    bk = np.asarray(bk, dtype=np.float32)
    Wv = np.asarray(Wv, dtype=np.float32)
    bv = np.asarray(bv, dtype=np.float32)

    nc = _get_nc(S)
    in_maps = _shard_inputs(x, Wq, bq, Wk, bk, Wv, bv)
    try:
        res = run_bass_kernel_spmd(nc, in_maps, core_ids=list(range(8)),
                                   trace=_trace)
    except (ModuleNotFoundError, ImportError):
        # Tracing was requested (arg or BASS_TRACE env) but this axon client
        # has no NTFF profiling hooks -- rerun untraced.
        import os
        os.environ["BASS_NEVER_TRACE"] = "1"
        res = run_bass_kernel_spmd(nc, in_maps, core_ids=list(range(8)),
                                   trace=False)
    kernel._last_results = res

    out = np.empty((B, S, D), dtype=np.float32)
    for c in range(8):
        b, hp = divmod(c, 4)
        out[b, :, 128 * hp:128 * hp + 128] = res.results[c]["out"]
    return out


# revision 13
# speedup vs baseline: 1.0018x; 1.0008x over previous
"""Multi-head attention (B=2, S=4096, D=512, H=8, HD=64, fp32) on 8 TRN2 cores.

The softmax exp -- the original bottleneck (ScalarE ~252us busy of a ~313us
kernel) -- is split between ScalarE (true exp) and the Vector engine
(single-instruction Schraudolph exp2: i16 = rint(s*log2e*128 + 16250.5)
written through a bf16 tile's int16 bitcast view; the bf16 bit pattern IS
2^(s*log2e) to +-3.2%).  The softmax denominator is computed from the same
approximated values (ones column in v_sb), so the sawtooth's mean cancels;
measured output error 0.0046 vs the 2e-2 gate.  HW-validated: the DVE
f32->int16 conversion rounds to nearest (99.93% bit-exact vs rint).

Attention loop (CoreSim: PE 243.8us busy = 96-98.5% of the 247.7us marginal
/ 253.8us single-shot):
  - per kc chunk, BOTH heads' score matmuls are emitted back-to-back: their
    contraction rows (PE rows 0-63 / 64-127) are disjoint, so hardware that
    overlaps row-disjoint matmuls runs them concurrently (cost-model
    neutral, free upside on HW).
  - exp per (kc, head): head0 -> ScalarE, head1 -> DVE, with a few head1
    units flipped back to ScalarE to balance (r_dve); PSUM score tiles
    [128, 512] in a 6-deep rotation keep both engines fed.
  - each head's av->SBUF copy (outt: h0 ScalarE, h1 DVE) is emitted inline
    right after that head's accumulation stops, so the next q-group's
    attnV gets its PSUM slot back with minimal boundary stall.
  - bv is folded into the tail (out = attn@v/denom + bv via one DVE
    scalar_tensor_tensor) instead of pre-added into V; V writeback is a
    plain PSUM->SBUF bf16 copy on DVE, emitted lazily inside q-group 0.
  - the previous q-group's transpose/normalize/store tail is deferred
    tail_lag steps INTO the next q-group and spread one 128-row block per 3
    steps (no DVE burst); weights load as one combined DMA
    per projection (rearranged DRAM AP) on the Activation HWDGE queue while
    xT streams on the SP queue; qt/kt/v_sb are double-buffered by body
    parity (removes the WAR serialization between repeated bodies).

Sharding: core c -> batch b = c//4, head pair hp = c%4; attention is
head-independent so there is no cross-core communication.
"""

import numpy as np

B, S, D, H = 2, 4096, 512, 8
HD = D // H          # 64
OD = 128             # output dims per core (2 heads)
QW = 512             # query group width

LOG2E_128 = 128.0 / float(np.log(2.0))
BMAGIC = 16250.5     # 127*128 - 5.5 (sawtooth centering, round-to-nearest)

_CACHE = {}


def _build(s=S, rep=1, r_dve=0.42, bias_eng="S", wb_pat="D", tail_lag=8,
           outt_eng="SD", gcols=512, proj_at=2):
    import concourse.bacc as bacc
    import concourse.mybir as mybir
    import concourse.tile as tile

    f32 = mybir.dt.float32
    bf16 = mybir.dt.bfloat16
    i16 = mybir.dt.int16
    Exp = mybir.ActivationFunctionType.Exp
    Copy = mybir.ActivationFunctionType.Copy
    Ident = mybir.ActivationFunctionType.Identity
    Mult = mybir.AluOpType.mult
    Add = mybir.AluOpType.add

    qg_n = s // QW
    kc_n = s // 128
    sb_n = s // QW
    upg = gcols // QW     # units (kc,h) per exp group: 1 or 2
    grp_n = 2 * kc_n // upg
    n_dve = int(round(grp_n * r_dve))

    nc = bacc.Bacc(None, target_bir_lowering=False)

    xT = nc.dram_tensor("xT", [D, s], bf16, kind="ExternalInput")
    wqT = nc.dram_tensor("wqT", [D, OD], bf16, kind="ExternalInput")
    wkT = nc.dram_tensor("wkT", [D, OD], bf16, kind="ExternalInput")
    wvT = nc.dram_tensor("wvT", [D, OD], bf16, kind="ExternalInput")
    bq = nc.dram_tensor("bq", [OD, 1], f32, kind="ExternalInput")
    bk = nc.dram_tensor("bk", [OD, 1], f32, kind="ExternalInput")
    bvb = nc.dram_tensor("bvb", [128, OD], f32, kind="ExternalInput")
    ident = nc.dram_tensor("ident", [128, 128], f32, kind="ExternalInput")
    out = nc.dram_tensor("out", [s, OD], f32, kind="ExternalOutput")

    with tile.TileContext(nc) as tc:
        with (
            tc.tile_pool(name="persist", bufs=1) as persist,
            tc.tile_pool(name="exps", bufs=6) as exps,
            tc.tile_pool(name="outsb", bufs=8) as outsb,
            tc.tile_pool(name="outt", bufs=2) as outtp,
            tc.tile_pool(name="psbig", bufs=(3 if gcols == 1024 else 6),
                         space="PSUM") as psbig,
            tc.tile_pool(name="pssmall", bufs=2, space="PSUM") as pssmall,
        ):
            xt_all = persist.tile([128, 4, s], bf16, name="xt_all",
                                  tag="xt")
            xt = [xt_all[:, c, :] for c in range(4)]
            wk_a = persist.tile([128, 4, OD], bf16, name="wk_a", tag="wk")
            wq_a = persist.tile([128, 4, OD], bf16, name="wq_a", tag="wq")
            wv_a = persist.tile([128, 4, OD], bf16, name="wv_a", tag="wv")
            wq = [wq_a[:, c, :] for c in range(4)]
            wk = [wk_a[:, c, :] for c in range(4)]
            wv = [wv_a[:, c, :] for c in range(4)]
            bq_t = persist.tile([OD, 1], f32, name="bq_t", tag="bq")
            bk_t = persist.tile([OD, 1], f32, name="bk_t", tag="bk")
            bvb_t = persist.tile([128, OD], f32, name="bvb_t", tag="bvb")
            id_t = persist.tile([128, 128], f32, name="id_t", tag="ident")
            # Two HWDGE queues. The first projection block is ONE combined
            # DMA (all 4 x-chunks via a rearranged DRAM AP) on the SP queue
            # while wk loads on the Activation queue, so the first matmul's
            # inputs land in ~2us; the xT bulk streams after.
            # Warm the ScalarE exp table early: the
            # ~1.3us ACT_TABLE_LOAD otherwise delays the issue (both run on
            # the Activation engine) and with it the first matmul.
            warm = persist.tile([1, 1], f32, name="warm", tag="warm")
            nc.vector.memset(warm[:], 0.0)
            nc.scalar.activation(warm[:], warm[:], Exp)
            # All four chunks' first block as ONE 3D-AP DMA (one issue
            # overhead instead of four); the bulk stays fine-grained so the
            # K projection's consumption order is fed progressively.
            xTr = xT[:].rearrange("(c p) j -> p c j", p=128)
            nc.scalar.dma_start(
                wk_a[:], wkT[:].rearrange("(c p) j -> p c j", p=128))
            nc.sync.dma_start(xt_all[:, :, 0:QW], xTr[:, :, 0:QW])
            nc.sync.dma_start(bk_t[:], bk[:])
            nc.scalar.dma_start(
                wq_a[:], wqT[:].rearrange("(c p) j -> p c j", p=128))
            nc.scalar.dma_start(bq_t[:], bq[:])
            nc.scalar.dma_start(
                wv_a[:], wvT[:].rearrange("(c p) j -> p c j", p=128))
            nc.scalar.dma_start(bvb_t[:], bvb[:])
            nc.scalar.dma_start(id_t[:], ident[:])
            h2 = (s - QW) // 2 + QW
            for lo, hi in ((QW, h2), (h2, s)):
                for c in range(4):
                    nc.sync.dma_start(xt[c][:, lo:hi],
                                      xT[c * 128:(c + 1) * 128, lo:hi])

            # Double-buffered by body parity: repeated bodies otherwise
            # serialize on the WAR hazard (body N+1's projections overwrite
            # qt/kt/v_sb while body N's last q-group still reads them).
            qt2 = [persist.tile([128, s], bf16, name=f"qt{p}", tag=f"qt{p}")
                   for p in (0, 1)]
            kt2 = [persist.tile([128, s], bf16, name=f"kt{p}", tag=f"kt{p}")
                   for p in (0, 1)]
            v_sb2 = [[persist.tile([128, kc_n * 65], bf16,
                                   name=f"vsb{h}_{p}", tag=f"vsb{h}_{p}")
                      for h in (0, 1)] for p in (0, 1)]

            for r in range(rep):
                p = r % 2
                ctx = dict(nc=nc, mybir=mybir, s=s, qt=qt2[p], kt=kt2[p],
                           xt=xt, wq=wq, wk=wk, wv=wv, bq_t=bq_t, bk_t=bk_t,
                           bvb_t=bvb_t, id_t=id_t, v_sb=v_sb2[p], out=out,
                           exps=exps, outsb=outsb, outtp=outtp,
                           psbig=psbig, pssmall=pssmall,
                           n_dve=n_dve, bias_eng=bias_eng, wb_pat=wb_pat,
                           tail_lag=tail_lag, outt_eng=outt_eng, gcols=gcols,
                           proj_at=proj_at,
                           f32=f32, bf16=bf16, i16=i16, Exp=Exp, Copy=Copy,
                           Ident=Ident, Mult=Mult, Add=Add)
                _emit_body(ctx)

    nc.compile()
    return nc


def _emit_body(c):
    nc = c["nc"]
    s = c["s"]
    f32, bf16, i16 = c["f32"], c["bf16"], c["i16"]
    Exp, Copy, Ident, Mult, Add = (c["Exp"], c["Copy"], c["Ident"],
                                   c["Mult"], c["Add"])
    qt, kt, xt, v_sb = c["qt"], c["kt"], c["xt"], c["v_sb"]
    wq, wk, wv = c["wq"], c["wk"], c["wv"]
    bq_t, bk_t, bvb_t, id_t = c["bq_t"], c["bk_t"], c["bvb_t"], c["id_t"]
    out = c["out"]
    exps, outsb, outtp = c["exps"], c["outsb"], c["outtp"]
    psbig, pssmall = c["psbig"], c["pssmall"]
    n_dve, bias_eng, wb_pat = c["n_dve"], c["bias_eng"], c["wb_pat"]
    tail_lag, outt_eng = c["tail_lag"], c["outt_eng"]
    gcols = c["gcols"]

    qg_n = s // QW
    kc_n = s // 128
    sb_n = s // QW
    upg = gcols // QW
    grp_n = 2 * kc_n // upg

    # Exp engine assignment per (kc, head): head0 -> ScalarE, head1 -> DVE
    # (concurrent engines per kc pair); `flip_s` of the head1 units are
    # flipped back to ScalarE to fine-tune the load split.  The last two kc
    # run [h0,h0,h1,h1] with engines [S,S,D,D] so each head's av->SBUF copy
    # (outt: h0 ScalarE, h1 DVE) never queues behind the other engine.
    flip_s = max(0, 32 - n_dve) if upg == 1 else 0
    flip_set = set()
    acc = 0
    for kc2 in range(2, kc_n - 2):
        acc += flip_s
        if acc >= kc_n - 4:
            acc -= kc_n - 4
            flip_set.add(kc2)

    def exp_eng(kc2, h):
        if kc2 >= kc_n - 2:
            return "S" if h == 0 else "D"
        if h == 0:
            return "S"
        return "S" if kc2 in flip_set else "D"

    def proj_qk(dst, w, b_t, sb, narrow=False):
        ps = psbig.tile([128, QW], f32, name="ps_proj", tag="sc")
        # narrow: 128-col sub-blocks so the very first matmul only needs the
        # first 128 x columns (cuts the prologue DMA wait).
        subs = 4 if narrow else 1
        sw = QW // subs
        for sub in range(subs):
            for cc in range(4):
                nc.tensor.matmul(
                    ps[:, sub * sw:(sub + 1) * sw],
                    lhsT=w[cc][:],
                    rhs=xt[cc][:, sb * QW + sub * sw:sb * QW + (sub + 1) * sw],
                    start=(cc == 0),
                    stop=(cc == 3),
                )
        dslice = dst[:, sb * QW:(sb + 1) * QW]
        if bias_eng == "S":
            nc.scalar.activation(dslice, ps[:, 0:QW], Ident, bias=b_t[:])
        else:
            nc.vector.tensor_scalar_add(dslice, ps[:, 0:QW], b_t[:])

    # K fully, then Q block 0 (enough to start attention qg 0)
    for sb in range(sb_n):
        proj_qk(kt, wk, bk_t, sb)
    proj_qk(qt, wq, bq_t, 0)

    # ---- V projection: pure V (bv folded into the tail), ones col 64.
    # Emitted lazily, interleaved into q-group 0's unit loop (chunk kc lands
    # a few groups before unit (kc, h) consumes it) so the PE isn't serial
    # on V while the exp engines sit idle.
    for h in (0, 1):
        nc.vector.memset(v_sb[h][:], 1.0)

    vp_next = [0]

    def emit_vp_upto(kc_needed):
        while vp_next[0] <= min(kc_needed, kc_n - 1):
            sb = vp_next[0]
            ps = psbig.tile([128, 128], f32, name="ps_vp", tag="sc")
            for cc in range(4):
                nc.tensor.matmul(
                    ps[:],
                    lhsT=xt[cc][:, sb * 128:(sb + 1) * 128],
                    rhs=wv[cc][:],
                    start=(cc == 0),
                    stop=(cc == 3),
                )
            for h in (0, 1):
                dst = v_sb[h][:, sb * 65:sb * 65 + 64]
                src = ps[:, h * 64:(h + 1) * 64]
                eng = wb_pat[(2 * sb + h) % len(wb_pat)]
                if eng == "S":
                    nc.scalar.activation(dst, src, Copy)
                else:
                    nc.vector.tensor_copy(dst, src)
            vp_next[0] += 1

    # ---- attention ----
    # Tail split: the av->SBUF copies (which free the av PSUM slots for the
    # next q-group) are emitted right after the q-group's last attnV; the
    # transpose/normalize/store half is deferred into the next q-group so
    # both exp engines stay fed across the boundary.
    def emit_tail_block(qg, outts, blk, alt_q=False):
        ot = outsb.tile([128, OD], f32, name="ot", tag="outsb")
        for h in (0, 1):
            tp = psbig.tile([128, 65], f32, name="tp", tag="sc")
            nc.tensor.transpose(
                tp[:],
                outts[h][:, blk * 128:(blk + 1) * 128],
                id_t[0:65, 0:65],
            )
            rs = outsb.tile([128, 1], f32, name="rs", tag="rs")
            nc.vector.reciprocal(rs[:], tp[:, 64:65])
            nc.vector.scalar_tensor_tensor(
                ot[:, h * HD:(h + 1) * HD],
                tp[:, 0:64], rs[:],
                bvb_t[:, h * HD:(h + 1) * HD],
                Mult, Add,
            )
        r0 = qg * QW + blk * 128
        # alt_q: only for the kernel-final tail, when the Activation queue
        # is idle (mid-kernel its DMA issues would block ScalarE exps).
        eng = nc.scalar if (alt_q and blk % 2 == 1) else nc.sync
        eng.dma_start(out[r0:r0 + 128, :], ot[:])

    def emit_tail_rest(qg, outts, alt_q=False):
        for blk in range(4):
            emit_tail_block(qg, outts, blk, alt_q=alt_q)

    def emit_tail_half(qg, outt, h):
        # Last q-group only: per-head eager tail with half-width stores so
        # head 0's drain overlaps head 1's final units.
        for blk in range(4):
            tp = psbig.tile([128, 65], f32, name="tp", tag="sc")
            nc.tensor.transpose(
                tp[:],
                outt[:, blk * 128:(blk + 1) * 128],
                id_t[0:65, 0:65],
            )
            rs = outsb.tile([128, 1], f32, name="rs", tag="rs")
            oth = outsb.tile([128, HD], f32, name="oth", tag="outsb")
            nc.vector.reciprocal(rs[:], tp[:, 64:65])
            nc.vector.scalar_tensor_tensor(
                oth[:], tp[:, 0:64], rs[:],
                bvb_t[:, h * HD:(h + 1) * HD],
                Mult, Add,
            )
            r0 = qg * QW + blk * 128
            nc.sync.dma_start(
                out[r0:r0 + 128, h * HD:(h + 1) * HD], oth[:])

    def emit_one_outt(av, h):
        outt = outtp.tile([65, QW], f32, name="outt", tag="outt")
        if outt_eng[h] == "S":
            nc.scalar.activation(outt[:], av[h][:], Copy)
        else:
            nc.vector.tensor_copy(outt[:], av[h][:])
        return outt

    pending_tail = None
    for qg in range(qg_n):
        av = [pssmall.tile([65, QW], f32, name="av", tag="av")
              for _ in (0, 1)]
        outts = [None, None]
        plan = [(kc2, (0, 1)) for kc2 in range(kc_n)]
        for step, (kc2, heads) in enumerate(plan):
            if qg == 0:
                emit_vp_upto(kc2 + 3)
            if step == c.get("proj_at", 2) and qg + 1 < qg_n:
                proj_qk(qt, wq, bq_t, qg + 1)
            # Previous q-group's tail, one block per slot: spreads the
            # DVE recip/normalize burst across several steps.
            if (pending_tail is not None and step >= tail_lag
                    and (step - tail_lag) % 3 == 0):
                blk = (step - tail_lag) // 3
                emit_tail_block(pending_tail[0], pending_tail[1], blk)
                if blk == 3:
                    pending_tail = None
            pss = []
            for h in heads:
                ps = psbig.tile([128, QW], f32, name="ps_sc", tag="sc")
                nc.tensor.matmul(
                    ps[:],
                    lhsT=kt[h * HD:(h + 1) * HD,
                            kc2 * 128:(kc2 + 1) * 128],
                    rhs=qt[h * HD:(h + 1) * HD, qg * QW:(qg + 1) * QW],
                    start=True,
                    stop=True,
                )
                pss.append(ps)
            exs = []
            for ps, h in zip(pss, heads):
                ex = exps.tile([128, QW], bf16, name="ex", tag="exp")
                if exp_eng(kc2, h) == "D":
                    nc.vector.tensor_scalar(
                        ex[:].bitcast(i16), ps[:], LOG2E_128, BMAGIC,
                        Mult, Add)
                else:
                    nc.scalar.activation(ex[:], ps[:], Exp)
                exs.append(ex)
            for ex, h in zip(exs, heads):
                nc.tensor.matmul(
                    av[h][:],
                    lhsT=v_sb[h][:, kc2 * 65:kc2 * 65 + 65],
                    rhs=ex[:],
                    start=(kc2 == 0),
                    stop=(kc2 == kc_n - 1),
                )
                if kc2 == kc_n - 1:
                    outts[h] = emit_one_outt(av, h)
        if pending_tail is not None:
            emit_tail_rest(*pending_tail)
            pending_tail = None
        pending_tail = (qg, outts)
    emit_tail_rest(*pending_tail, alt_q=True)
                pending_tail = None
            seq = unit_seq_last if qg == qg_n - 1 else unit_seq
            units = seq[g * upg:(g + 1) * upg]
            ps = psbig.tile([128, upg * QW], f32, name="ps_sc", tag="sc")
            for i, (kc, h) in enumerate(units):
                nc.tensor.matmul(
                    ps[:, i * QW:(i + 1) * QW],
                    lhsT=kt[h * HD:(h + 1) * HD, kc * 128:(kc + 1) * 128],
                    rhs=qt[h * HD:(h + 1) * HD, qg * QW:(qg + 1) * QW],
                    start=True,
                    stop=True,
                )
            ex = exps.tile([128, upg * QW], bf16, name="ex", tag="exp")
            if g in dve_set:
                nc.vector.tensor_scalar(
                    ex[:].bitcast(i16), ps[:], LOG2E_128, BMAGIC, Mult, Add)
            else:
                nc.scalar.activation(ex[:], ps[:], Exp)
            for i, (kc, h) in enumerate(units):
                nc.tensor.matmul(
                    av[h][:],
                    lhsT=v_sb[h][:, kc * 65:kc * 65 + 65],
                    rhs=ex[:, i * QW:(i + 1) * QW],
                    start=(kc == 0),
                    stop=(kc == kc_n - 1),
                )
                if kc == kc_n - 1:
                    outts[h] = emit_one_outt(av, h)
        if pending_tail is not None:
            emit_tail_rest(*pending_tail)
            pending_tail = None
        pending_tail = (qg, outts)
    emit_tail_rest(*pending_tail, alt_q=True)


def _get_nc(s=S):
    if s not in _CACHE:
        _CACHE[s] = _build(s)
    return _CACHE[s]


def _shard_inputs(x, Wq, bq, Wk, bk, Wv, bv):
    import ml_dtypes

    bf16 = ml_dtypes.bfloat16
    f32 = np.float32
    ident = np.eye(128, dtype=f32)
    xTb = [np.ascontiguousarray(x[b].T).astype(bf16) for b in range(B)]
    wq_s, wk_s, wv_s, bq_s, bk_s, bvb_s = [], [], [], [], [], []
    for hp in range(4):
        r = slice(128 * hp, 128 * hp + 128)
        wq_s.append(np.ascontiguousarray((Wq[r] * 0.125).T).astype(bf16))
        wk_s.append(np.ascontiguousarray(Wk[r].T).astype(bf16))
        wv_s.append(np.ascontiguousarray(Wv[r].T).astype(bf16))
        bq_s.append((bq[r] * 0.125).reshape(128, 1).astype(f32))
        bk_s.append(bk[r].reshape(128, 1).astype(f32))
        bvb_s.append(np.tile(bv[r][None, :], (128, 1)).astype(f32))
    in_maps = []
    for c in range(8):
        b, hp = divmod(c, 4)
        in_maps.append({
            "xT": xTb[b],
            "wqT": wq_s[hp],
            "wkT": wk_s[hp],
            "wvT": wv_s[hp],
            "bq": bq_s[hp],
            "bk": bk_s[hp],
            "bvb": bvb_s[hp],
            "ident": ident,
        })
    return in_maps


def kernel(x, Wq, bq, Wk, bk, Wv, bv, _trace=False):
    from concourse.bass_utils import run_bass_kernel_spmd

    x = np.asarray(x, dtype=np.float32)
    Wq = np.asarray(Wq, dtype=np.float32)
    bq = np.asarray(bq, dtype=np.float32)
    Wk = np.asarray(Wk, dtype=np.float32)
    bk = np.asarray(bk, dtype=np.float32)
    Wv = np.asarray(Wv, dtype=np.float32)
    bv = np.asarray(bv, dtype=np.float32)

    nc = _get_nc(S)
    in_maps = _shard_inputs(x, Wq, bq, Wk, bk, Wv, bv)
    try:
        res = run_bass_kernel_spmd(nc, in_maps, core_ids=list(range(8)),
                                   trace=_trace)
    except (ModuleNotFoundError, ImportError):
        # Tracing was requested (arg or BASS_TRACE env) but this axon client
        # has no NTFF profiling hooks -- rerun untraced.
        import os
        os.environ["BASS_NEVER_TRACE"] = "1"
        res = run_bass_kernel_spmd(nc, in_maps, core_ids=list(range(8)),
                                   trace=False)
    kernel._last_results = res

    out = np.empty((B, S, D), dtype=np.float32)
    for c in range(8):
        b, hp = divmod(c, 4)
        out[b, :, 128 * hp:128 * hp + 128] = res.results[c]["out"]
    return out


# revision 14
# speedup vs baseline: 1.0022x; 1.0004x over previous
"""Multi-head attention (B=2, S=4096, D=512, H=8, HD=64, fp32) on 8 TRN2 cores.

The softmax exp -- the original bottleneck (ScalarE ~252us busy of a ~313us
kernel) -- is split between ScalarE (true exp) and the Vector engine
(single-instruction Schraudolph exp2: i16 = rint(s*log2e*128 + 16250.5)
written through a bf16 tile's int16 bitcast view; the bf16 bit pattern IS
2^(s*log2e) to +-3.2%).  The softmax denominator is computed from the same
approximated values (ones column in v_sb), so the sawtooth's mean cancels;
measured output error 0.0046 vs the 2e-2 gate.  HW-validated: the DVE
f32->int16 conversion rounds to nearest (99.93% bit-exact vs rint).

Attention loop (CoreSim: PE 243.8us busy = 96-98.5% of the 247.7us marginal
/ 253.8us single-shot):
  - per kc chunk, BOTH heads' score matmuls are emitted back-to-back: their
    contraction rows (PE rows 0-63 / 64-127) are disjoint, so hardware that
    overlaps row-disjoint matmuls runs them concurrently (cost-model
    neutral, free upside on HW).
  - exp per (kc, head): head0 -> ScalarE, head1 -> DVE, with a few head1
    units flipped back to ScalarE to balance (r_dve); PSUM score tiles
    [128, 512] in a 6-deep rotation keep both engines fed.
  - each head's av->SBUF copy (outt: h0 ScalarE, h1 DVE) is emitted inline
    right after that head's accumulation stops, so the next q-group's
    attnV gets its PSUM slot back with minimal boundary stall.
  - bv is folded into the tail (out = attn@v/denom + bv via one DVE
    scalar_tensor_tensor) instead of pre-added into V; V writeback is a
    plain PSUM->SBUF bf16 copy on DVE, emitted lazily inside q-group 0.
  - the previous q-group's transpose/normalize/store tail is deferred
    tail_lag steps INTO the next q-group and spread one 128-row block per 3
    steps (no DVE burst); weights load as one combined DMA
    per projection (rearranged DRAM AP) on the Activation HWDGE queue while
    xT streams on the SP queue; qt/kt/v_sb are double-buffered by body
    parity (removes the WAR serialization between repeated bodies).

Sharding: core c -> batch b = c//4, head pair hp = c%4; attention is
head-independent so there is no cross-core communication.
"""

import numpy as np

B, S, D, H = 2, 4096, 512, 8
HD = D // H          # 64
OD = 128             # output dims per core (2 heads)
QW = 512             # query group width

LOG2E_128 = 128.0 / float(np.log(2.0))
BMAGIC = 16250.5     # 127*128 - 5.5 (sawtooth centering, round-to-nearest)

_CACHE = {}


def _build(s=S, rep=1, r_dve=0.44, bias_eng="S", wb_pat="D", tail_lag=10,
           outt_eng="SD", gcols=512, proj_at=2):
    import concourse.bacc as bacc
    import concourse.mybir as mybir
    import concourse.tile as tile

    f32 = mybir.dt.float32
    bf16 = mybir.dt.bfloat16
    i16 = mybir.dt.int16
    Exp = mybir.ActivationFunctionType.Exp
    Copy = mybir.ActivationFunctionType.Copy
    Ident = mybir.ActivationFunctionType.Identity
    Mult = mybir.AluOpType.mult
    Add = mybir.AluOpType.add

    qg_n = s // QW
    kc_n = s // 128
    sb_n = s // QW
    upg = gcols // QW     # units (kc,h) per exp group: 1 or 2
    grp_n = 2 * kc_n // upg
    n_dve = int(round(grp_n * r_dve))

    nc = bacc.Bacc(None, target_bir_lowering=False)

    xT = nc.dram_tensor("xT", [D, s], bf16, kind="ExternalInput")
    wqT = nc.dram_tensor("wqT", [D, OD], bf16, kind="ExternalInput")
    wkT = nc.dram_tensor("wkT", [D, OD], bf16, kind="ExternalInput")
    wvT = nc.dram_tensor("wvT", [D, OD], bf16, kind="ExternalInput")
    bq = nc.dram_tensor("bq", [OD, 1], f32, kind="ExternalInput")
    bk = nc.dram_tensor("bk", [OD, 1], f32, kind="ExternalInput")
    bvb = nc.dram_tensor("bvb", [128, OD], f32, kind="ExternalInput")
    ident = nc.dram_tensor("ident", [128, 128], f32, kind="ExternalInput")
    out = nc.dram_tensor("out", [s, OD], f32, kind="ExternalOutput")

    with tile.TileContext(nc) as tc:
        with (
            tc.tile_pool(name="persist", bufs=1) as persist,
            tc.tile_pool(name="exps", bufs=6) as exps,
            tc.tile_pool(name="outsb", bufs=8) as outsb,
            tc.tile_pool(name="outt", bufs=2) as outtp,
            tc.tile_pool(name="psbig", bufs=(3 if gcols == 1024 else 6),
                         space="PSUM") as psbig,
            tc.tile_pool(name="pssmall", bufs=2, space="PSUM") as pssmall,
        ):
            xt_all = persist.tile([128, 4, s], bf16, name="xt_all",
                                  tag="xt")
            xt = [xt_all[:, c, :] for c in range(4)]
            wk_a = persist.tile([128, 4, OD], bf16, name="wk_a", tag="wk")
            wq_a = persist.tile([128, 4, OD], bf16, name="wq_a", tag="wq")
            wv_a = persist.tile([128, 4, OD], bf16, name="wv_a", tag="wv")
            wq = [wq_a[:, c, :] for c in range(4)]
            wk = [wk_a[:, c, :] for c in range(4)]
            wv = [wv_a[:, c, :] for c in range(4)]
            bq_t = persist.tile([OD, 1], f32, name="bq_t", tag="bq")
            bk_t = persist.tile([OD, 1], f32, name="bk_t", tag="bk")
            bvb_t = persist.tile([128, OD], f32, name="bvb_t", tag="bvb")
            id_t = persist.tile([128, 128], f32, name="id_t", tag="ident")
            # Two HWDGE queues. The first projection block is ONE combined
            # DMA (all 4 x-chunks via a rearranged DRAM AP) on the SP queue
            # while wk loads on the Activation queue, so the first matmul's
            # inputs land in ~2us; the xT bulk streams after.
            # Warm the ScalarE exp table early: the
            # ~1.3us ACT_TABLE_LOAD otherwise delays the issue (both run on
            # the Activation engine) and with it the first matmul.
            warm = persist.tile([1, 1], f32, name="warm", tag="warm")
            nc.vector.memset(warm[:], 0.0)
            nc.scalar.activation(warm[:], warm[:], Exp)
            # All four chunks' first block as ONE 3D-AP DMA (one issue
            # overhead instead of four); the bulk stays fine-grained so the
            # K projection's consumption order is fed progressively.
            xTr = xT[:].rearrange("(c p) j -> p c j", p=128)
            nc.scalar.dma_start(
                wk_a[:], wkT[:].rearrange("(c p) j -> p c j", p=128))
            nc.sync.dma_start(xt_all[:, :, 0:QW], xTr[:, :, 0:QW])
            nc.sync.dma_start(bk_t[:], bk[:])
            nc.scalar.dma_start(
                wq_a[:], wqT[:].rearrange("(c p) j -> p c j", p=128))
            nc.scalar.dma_start(bq_t[:], bq[:])
            nc.scalar.dma_start(
                wv_a[:], wvT[:].rearrange("(c p) j -> p c j", p=128))
            nc.scalar.dma_start(bvb_t[:], bvb[:])
            nc.scalar.dma_start(id_t[:], ident[:])
            h2 = (s - QW) // 2 + QW
            for lo, hi in ((QW, h2), (h2, s)):
                for c in range(4):
                    nc.sync.dma_start(xt[c][:, lo:hi],
                                      xT[c * 128:(c + 1) * 128, lo:hi])

            # Double-buffered by body parity: repeated bodies otherwise
            # serialize on the WAR hazard (body N+1's projections overwrite
            # qt/kt/v_sb while body N's last q-group still reads them).
            qt2 = [persist.tile([128, s], bf16, name=f"qt{p}", tag=f"qt{p}")
                   for p in (0, 1)]
            kt2 = [persist.tile([128, s], bf16, name=f"kt{p}", tag=f"kt{p}")
                   for p in (0, 1)]
            v_sb2 = [[persist.tile([128, kc_n * 65], bf16,
                                   name=f"vsb{h}_{p}", tag=f"vsb{h}_{p}")
                      for h in (0, 1)] for p in (0, 1)]

            for r in range(rep):
                p = r % 2
                ctx = dict(nc=nc, mybir=mybir, s=s, qt=qt2[p], kt=kt2[p],
                           xt=xt, wq=wq, wk=wk, wv=wv, bq_t=bq_t, bk_t=bk_t,
                           bvb_t=bvb_t, id_t=id_t, v_sb=v_sb2[p], out=out,
                           exps=exps, outsb=outsb, outtp=outtp,
                           psbig=psbig, pssmall=pssmall,
                           n_dve=n_dve, bias_eng=bias_eng, wb_pat=wb_pat,
                           tail_lag=tail_lag, outt_eng=outt_eng, gcols=gcols,
                           proj_at=proj_at,
                           f32=f32, bf16=bf16, i16=i16, Exp=Exp, Copy=Copy,
                           Ident=Ident, Mult=Mult, Add=Add)
                _emit_body(ctx)

    nc.compile()
    return nc


def _emit_body(c):
    nc = c["nc"]
    s = c["s"]
    f32, bf16, i16 = c["f32"], c["bf16"], c["i16"]
    Exp, Copy, Ident, Mult, Add = (c["Exp"], c["Copy"], c["Ident"],
                                   c["Mult"], c["Add"])
    qt, kt, xt, v_sb = c["qt"], c["kt"], c["xt"], c["v_sb"]
    wq, wk, wv = c["wq"], c["wk"], c["wv"]
    bq_t, bk_t, bvb_t, id_t = c["bq_t"], c["bk_t"], c["bvb_t"], c["id_t"]
    out = c["out"]
    exps, outsb, outtp = c["exps"], c["outsb"], c["outtp"]
    psbig, pssmall = c["psbig"], c["pssmall"]
    n_dve, bias_eng, wb_pat = c["n_dve"], c["bias_eng"], c["wb_pat"]
    tail_lag, outt_eng = c["tail_lag"], c["outt_eng"]
    gcols = c["gcols"]

    qg_n = s // QW
    kc_n = s // 128
    sb_n = s // QW
    upg = gcols // QW
    grp_n = 2 * kc_n // upg

    # Exp engine assignment per (kc, head): head0 -> ScalarE, head1 -> DVE
    # (concurrent engines per kc pair); `flip_s` of the head1 units are
    # flipped back to ScalarE to fine-tune the load split.  The last two kc
    # run [h0,h0,h1,h1] with engines [S,S,D,D] so each head's av->SBUF copy
    # (outt: h0 ScalarE, h1 DVE) never queues behind the other engine.
    flip_s = max(0, 32 - n_dve) if upg == 1 else 0
    flip_set = set()
    acc = 0
    for kc2 in range(2, kc_n - 2):
        acc += flip_s
        if acc >= kc_n - 4:
            acc -= kc_n - 4
            flip_set.add(kc2)

    def exp_eng(kc2, h):
        if kc2 >= kc_n - 2:
            return "S" if h == 0 else "D"
        if h == 0:
            return "S"
        return "S" if kc2 in flip_set else "D"

    def proj_qk(dst, w, b_t, sb, narrow=False):
        ps = psbig.tile([128, QW], f32, name="ps_proj", tag="sc")
        # narrow: 128-col sub-blocks so the very first matmul only needs the
        # first 128 x columns (cuts the prologue DMA wait).
        subs = 4 if narrow else 1
        sw = QW // subs
        for sub in range(subs):
            for cc in range(4):
                nc.tensor.matmul(
                    ps[:, sub * sw:(sub + 1) * sw],
                    lhsT=w[cc][:],
                    rhs=xt[cc][:, sb * QW + sub * sw:sb * QW + (sub + 1) * sw],
                    start=(cc == 0),
                    stop=(cc == 3),
                )
        dslice = dst[:, sb * QW:(sb + 1) * QW]
        if bias_eng == "S":
            nc.scalar.activation(dslice, ps[:, 0:QW], Ident, bias=b_t[:])
        else:
            nc.vector.tensor_scalar_add(dslice, ps[:, 0:QW], b_t[:])

    # K fully, then Q block 0 (enough to start attention qg 0)
    for sb in range(sb_n):
        proj_qk(kt, wk, bk_t, sb)
    proj_qk(qt, wq, bq_t, 0)

    # ---- V projection: pure V (bv folded into the tail), ones col 64.
    # Emitted lazily, interleaved into q-group 0's unit loop (chunk kc lands
    # a few groups before unit (kc, h) consumes it) so the PE isn't serial
    # on V while the exp engines sit idle.
    for h in (0, 1):
        nc.vector.memset(v_sb[h][:], 1.0)

    vp_next = [0]

    def emit_vp_upto(kc_needed):
        while vp_next[0] <= min(kc_needed, kc_n - 1):
            sb = vp_next[0]
            ps = psbig.tile([128, 128], f32, name="ps_vp", tag="sc")
            for cc in range(4):
                nc.tensor.matmul(
                    ps[:],
                    lhsT=xt[cc][:, sb * 128:(sb + 1) * 128],
                    rhs=wv[cc][:],
                    start=(cc == 0),
                    stop=(cc == 3),
                )
            for h in (0, 1):
                dst = v_sb[h][:, sb * 65:sb * 65 + 64]
                src = ps[:, h * 64:(h + 1) * 64]
                eng = wb_pat[(2 * sb + h) % len(wb_pat)]
                if eng == "S":
                    nc.scalar.activation(dst, src, Copy)
                else:
                    nc.vector.tensor_copy(dst, src)
            vp_next[0] += 1

    # ---- attention ----
    # Tail split: the av->SBUF copies (which free the av PSUM slots for the
    # next q-group) are emitted right after the q-group's last attnV; the
    # transpose/normalize/store half is deferred into the next q-group so
    # both exp engines stay fed across the boundary.
    def emit_tail_block(qg, outts, blk, alt_q=False):
        ot = outsb.tile([128, OD], f32, name="ot", tag="outsb")
        for h in (0, 1):
            tp = psbig.tile([128, 65], f32, name="tp", tag="sc")
            nc.tensor.transpose(
                tp[:],
                outts[h][:, blk * 128:(blk + 1) * 128],
                id_t[0:65, 0:65],
            )
            rs = outsb.tile([128, 1], f32, name="rs", tag="rs")
            nc.vector.reciprocal(rs[:], tp[:, 64:65])
            nc.vector.scalar_tensor_tensor(
                ot[:, h * HD:(h + 1) * HD],
                tp[:, 0:64], rs[:],
                bvb_t[:, h * HD:(h + 1) * HD],
                Mult, Add,
            )
        r0 = qg * QW + blk * 128
        # alt_q: only for the kernel-final tail, when the Activation queue
        # is idle (mid-kernel its DMA issues would block ScalarE exps).
        eng = nc.scalar if (alt_q and blk % 2 == 1) else nc.sync
        eng.dma_start(out[r0:r0 + 128, :], ot[:])

    def emit_tail_rest(qg, outts, alt_q=False):
        for blk in range(4):
            emit_tail_block(qg, outts, blk, alt_q=alt_q)

    def emit_tail_half(qg, outt, h):
        # Last q-group only: per-head eager tail with half-width stores so
        # head 0's drain overlaps head 1's final units.
        for blk in range(4):
            tp = psbig.tile([128, 65], f32, name="tp", tag="sc")
            nc.tensor.transpose(
                tp[:],
                outt[:, blk * 128:(blk + 1) * 128],
                id_t[0:65, 0:65],
            )
            rs = outsb.tile([128, 1], f32, name="rs", tag="rs")
            oth = outsb.tile([128, HD], f32, name="oth", tag="outsb")
            nc.vector.reciprocal(rs[:], tp[:, 64:65])
            nc.vector.scalar_tensor_tensor(
                oth[:], tp[:, 0:64], rs[:],
                bvb_t[:, h * HD:(h + 1) * HD],
                Mult, Add,
            )
            r0 = qg * QW + blk * 128
            nc.sync.dma_start(
                out[r0:r0 + 128, h * HD:(h + 1) * HD], oth[:])

    def emit_one_outt(av, h):
        outt = outtp.tile([65, QW], f32, name="outt", tag="outt")
        if outt_eng[h] == "S":
            nc.scalar.activation(outt[:], av[h][:], Copy)
        else:
            nc.vector.tensor_copy(outt[:], av[h][:])
        return outt

    pending_tail = None
    for qg in range(qg_n):
        av = [pssmall.tile([65, QW], f32, name="av", tag="av")
              for _ in (0, 1)]
        outts = [None, None]
        plan = [(kc2, (0, 1)) for kc2 in range(kc_n)]
        for step, (kc2, heads) in enumerate(plan):
            if qg == 0:
                emit_vp_upto(kc2 + 3)
            if step == c.get("proj_at", 2) and qg + 1 < qg_n:
                proj_qk(qt, wq, bq_t, qg + 1)
            # Previous q-group's tail, one block per slot: spreads the
            # DVE recip/normalize burst across several steps.
            if (pending_tail is not None and step >= tail_lag
                    and (step - tail_lag) % 3 == 0):
                blk = (step - tail_lag) // 3
                emit_tail_block(pending_tail[0], pending_tail[1], blk)
                if blk == 3:
                    pending_tail = None
            pss = []
            for h in heads:
                ps = psbig.tile([128, QW], f32, name="ps_sc", tag="sc")
                nc.tensor.matmul(
                    ps[:],
                    lhsT=kt[h * HD:(h + 1) * HD,
                            kc2 * 128:(kc2 + 1) * 128],
                    rhs=qt[h * HD:(h + 1) * HD, qg * QW:(qg + 1) * QW],
                    start=True,
                    stop=True,
                )
                pss.append(ps)
            exs = []
            for ps, h in zip(pss, heads):
                ex = exps.tile([128, QW], bf16, name="ex", tag="exp")
                if exp_eng(kc2, h) == "D":
                    nc.vector.tensor_scalar(
                        ex[:].bitcast(i16), ps[:], LOG2E_128, BMAGIC,
                        Mult, Add)
                else:
                    nc.scalar.activation(ex[:], ps[:], Exp)
                exs.append(ex)
            for ex, h in zip(exs, heads):
                nc.tensor.matmul(
                    av[h][:],
                    lhsT=v_sb[h][:, kc2 * 65:kc2 * 65 + 65],
                    rhs=ex[:],
                    start=(kc2 == 0),
                    stop=(kc2 == kc_n - 1),
                )
                if kc2 == kc_n - 1:
                    outts[h] = emit_one_outt(av, h)
        if pending_tail is not None:
            emit_tail_rest(*pending_tail)
            pending_tail = None
        pending_tail = (qg, outts)
    emit_tail_rest(*pending_tail, alt_q=True)
                pending_tail = None
            seq = unit_seq_last if qg == qg_n - 1 else unit_seq
            units = seq[g * upg:(g + 1) * upg]
            ps = psbig.tile([128, upg * QW], f32, name="ps_sc", tag="sc")
            for i, (kc, h) in enumerate(units):
                nc.tensor.matmul(
                    ps[:, i * QW:(i + 1) * QW],
                    lhsT=kt[h * HD:(h + 1) * HD, kc * 128:(kc + 1) * 128],
                    rhs=qt[h * HD:(h + 1) * HD, qg * QW:(qg + 1) * QW],
                    start=True,
                    stop=True,
                )
            ex = exps.tile([128, upg * QW], bf16, name="ex", tag="exp")
            if g in dve_set:
                nc.vector.tensor_scalar(
                    ex[:].bitcast(i16), ps[:], LOG2E_128, BMAGIC, Mult, Add)
            else:
                nc.scalar.activation(ex[:], ps[:], Exp)
            for i, (kc, h) in enumerate(units):
                nc.tensor.matmul(
                    av[h][:],
                    lhsT=v_sb[h][:, kc * 65:kc * 65 + 65],
                    rhs=ex[:, i * QW:(i + 1) * QW],
                    start=(kc == 0),
                    stop=(kc == kc_n - 1),
                )
                if kc == kc_n - 1:
                    outts[h] = emit_one_outt(av, h)
        if pending_tail is not None:
            emit_tail_rest(*pending_tail)
            pending_tail = None
        pending_tail = (qg, outts)
    emit_tail_rest(*pending_tail, alt_q=True)


def _get_nc(s=S):
    if s not in _CACHE:
        _CACHE[s] = _build(s)
    return _CACHE[s]


def _shard_inputs(x, Wq, bq, Wk, bk, Wv, bv):
    import ml_dtypes

    bf16 = ml_dtypes.bfloat16
    f32 = np.float32
    ident = np.eye(128, dtype=f32)
    xTb = [np.ascontiguousarray(x[b].T).astype(bf16) for b in range(B)]
    wq_s, wk_s, wv_s, bq_s, bk_s, bvb_s = [], [], [], [], [], []
    for hp in range(4):
        r = slice(128 * hp, 128 * hp + 128)
        wq_s.append(np.ascontiguousarray((Wq[r] * 0.125).T).astype(bf16))
        wk_s.append(np.ascontiguousarray(Wk[r].T).astype(bf16))
        wv_s.append(np.ascontiguousarray(Wv[r].T).astype(bf16))
        bq_s.append((bq[r] * 0.125).reshape(128, 1).astype(f32))
        bk_s.append(bk[r].reshape(128, 1).astype(f32))
        bvb_s.append(np.tile(bv[r][None, :], (128, 1)).astype(f32))
    in_maps = []
    for c in range(8):
        b, hp = divmod(c, 4)
        in_maps.append({
            "xT": xTb[b],
            "wqT": wq_s[hp],
            "wkT": wk_s[hp],
            "wvT": wv_s[hp],
            "bq": bq_s[hp],
            "bk": bk_s[hp],
            "bvb": bvb_s[hp],
            "ident": ident,
        })
    return in_maps


def kernel(x, Wq, bq, Wk, bk, Wv, bv, _trace=False):
    from concourse.bass_utils import run_bass_kernel_spmd

    x = np.asarray(x, dtype=np.float32)
    Wq = np.asarray(Wq, dtype=np.float32)
    bq = np.asarray(bq, dtype=np.float32)
    Wk = np.asarray(Wk, dtype=np.float32)
    bk = np.asarray(bk, dtype=np.float32)
    Wv = np.asarray(Wv, dtype=np.float32)
    bv = np.asarray(bv, dtype=np.float32)

    nc = _get_nc(S)
    in_maps = _shard_inputs(x, Wq, bq, Wk, bk, Wv, bv)
    try:
        res = run_bass_kernel_spmd(nc, in_maps, core_ids=list(range(8)),
                                   trace=_trace)
    except (ModuleNotFoundError, ImportError):
        # Tracing was requested (arg or BASS_TRACE env) but this axon client
        # has no NTFF profiling hooks -- rerun untraced.
        import os
        os.environ["BASS_NEVER_TRACE"] = "1"
        res = run_bass_kernel_spmd(nc, in_maps, core_ids=list(range(8)),
                                   trace=False)
    kernel._last_results = res

    out = np.empty((B, S, D), dtype=np.float32)
    for c in range(8):
        b, hp = divmod(c, 4)
        out[b, :, 128 * hp:128 * hp + 128] = res.results[c]["out"]
    return out
